# revision 4
# baseline (speedup 1.0000x reference)
"""depth_to_space (DCR, block=2) on 8 NeuronCores.

out[b, 2h+i, 2w+j, c] = in[b, h, w, (2i+j)*64 + c]   for in [32,64,64,256] f32.

Sharding: batch dim B=32 split as 4 examples per core (data parallel, no
communication).

Per-core kernel: the permutation collapses to strided DRAM->DRAM DMA copies,
one per output-row parity i in {0,1}:
  - fuse (j,c) -> jc in [0,128): for fixed i the source slice
    x[:, :, :, i*128:(i+1)*128] merges (b,h,w) into a single stride dim:
    [[256, b*h*w], [1, 128]] (512B contiguous runs, 1KB stride);
  - the destination y[:, i::2, :, :] merges to [[16384, b*h], [1, 8192]]
    (output rows are fully contiguous).
No SBUF, no compute engines - pure DMA.

Engine assignment (measured, serialized device-loop timing on HW): parity 0
on the SP HWDGE ring (nc.sync) and parity 1 on the Activation HWDGE ring
(nc.scalar), with each parity's copy chunked into 4 row-ranges of 64 (b,h)
rows and lag-1 cross-semaphore waits between the rings.  The lockstep bounds
inter-queue drift so the SDMA packet round-robin interleaves the two
complementary 512B half-rows of each 1KB input row, keeping HBM reads
sequential.  Measured per-core serialized loop times (N=500/2000 device
loop): this scheme ~101.2us (~332 GB/s HBM R+W) vs 116.4us for the previous
SP+GPSIMD-SWDGE split, 111.0us for unchunked dual-ring, and a 100.5us
contiguous-memcpy ceiling (a single sequential stream; two concurrent
streams measure ~102us, so this sits at the achievable bound).  Chunk sizes
32/44/52/64/96/128 rows, lag-2, staggered boundaries, per-chunk parity
checkerboarding, per-row-pair chained DMAs (117us), 3-queue splits, and an
fp16-output SBUF pipeline (217us, DVE-cast-bound) all measured worse.
No GPSIMD/SWDGE work, so the Block skips the expensive gpsimd dge_drain.
"""

import numpy as np

import concourse.bass as bass
import concourse.mybir as mybir
from concourse.bass_utils import run_bass_kernel_spmd

B, H, W, C = 32, 64, 64, 256
KS = 2
OC = C // (KS * KS)
N_CORES = 8
BS = B // N_CORES
N_ROWS = BS * H  # 256 (b,h) row-pairs per core
ROWS_PER_CHUNK = 64
N_CHUNKS = N_ROWS // ROWS_PER_CHUNK

_nc_cache = None


def build_nc() -> bass.Bass:
    # no partition-id tensor or monotonic sems: trims unused per-execution
    # preamble work (runs once per NEFF execution, ahead of the DMAs)
    nc = bass.Bass(enable_partition_id=False, monotonic_sem_count=0)
    x = nc.declare_dram_parameter("x", [BS, H, W, C], mybir.dt.float32, isOutput=False)
    y = nc.declare_dram_parameter(
        "y", [BS, H * KS, W * KS, OC], mybir.dt.float32, isOutput=True
    )

    # src[:, i, :]: [[256, BS*H*W], [1, 128]] starting at element offset i*128
    src = x.rearrange("b h w (i jc) -> (b h w) i jc", i=KS)
    # dst[:, i, :]: [[16384, BS*H], [1, 8192]] starting at element offset i*8192
    dst = y.rearrange("b (h i) w c -> (b h) i (w c)", i=KS)

    K = ROWS_PER_CHUNK
    with nc.Block(no_gpsimd_drain=True) as block:
        s1 = nc.alloc_semaphore("dma_sem")
        s2 = nc.alloc_semaphore("dma_sem2")

        def prog(e, i, own, other):
            for k in range(N_CHUNKS):
                e.dma_start(
                    out=dst[k * K : (k + 1) * K, i, :],
                    in_=src[k * K * W : (k + 1) * K * W, i, :],
                ).then_inc(own, 16)
                if k >= 1:
                    # lag-1 lockstep with the other ring
                    e.wait_ge(other, 16 * k)
            e.wait_ge(own, 16 * N_CHUNKS)

        @block.sync
        def _(e):
            prog(e, 0, s1, s2)

        @block.scalar
        def _(e):
            prog(e, 1, s2, s1)

    return nc


def kernel(batch: np.ndarray) -> np.ndarray:
    global _nc_cache
    if _nc_cache is None:
        _nc_cache = build_nc()
    nc = _nc_cache

    batch = np.ascontiguousarray(np.asarray(batch), dtype=np.float32)
    assert batch.shape == (B, H, W, C), batch.shape

    in_maps = [{"x": batch[k * BS : (k + 1) * BS]} for k in range(N_CORES)]
    res = run_bass_kernel_spmd(nc, in_maps, list(range(N_CORES)))
    return np.concatenate([res.results[k]["y"] for k in range(N_CORES)], axis=0)
